# revision 2
# baseline (speedup 1.0000x reference)
"""Llama trunk (2 layers, before final norm) on 8 trn2 cores — v2.

Megatron tensor-parallel over 8 cores (4 q heads + 1 kv head, 1024 of
8192 FF dims per core).  Residual x lives on-chip in bf16, transposed
([DM, S]); all matmuls contract over the partition axis in bf16.
RMSNorm's rsqrt is broadcast via a K=1 matmul and applied during psum
evictions; gains and 1/sqrt(D) fold into weights on the host.  v comes
out of the combined k|v matmul and is PE-transposed to natural layout.
AllReduces run in bf16, split by sequence halves, issued early to
overlap attention/MLP compute.  All weight / collective staging DMAs
are batched into single multi-dim-AP transfers (per-dma_start dispatch
cost dominates otherwise).  Final down-proj partials reduce on host.
"""
import math
from contextlib import ExitStack

import ml_dtypes
import numpy as np

import concourse.bass as bass
import concourse.tile as tile
from concourse import bacc, masks, mybir
from concourse.alu_op_type import AluOpType
from concourse.bass_utils import run_bass_kernel_spmd

L, H, KVH, D = 2, 32, 8, 64
DM, FF = 2048, 8192
B, S = 1, 1024
EPS, THETA = 1e-5, 10000.0
NCORES = 8
QH = H // NCORES            # 4 q heads per core
QC = QH * D                 # 256 q cols per core
FFS = FF // NCORES          # 1024 ff dims per core
KT = DM // 128              # 16 contraction tiles over DM
NEG = -1.0e30

F32 = mybir.dt.float32
F32R = mybir.dt.float32r
BF16 = mybir.dt.bfloat16
AF = mybir.ActivationFunctionType


def _ap3(base_ap, offset_elems, ap):
    return bass.AP(tensor=base_ap.tensor, offset=base_ap.offset + offset_elems,
                   ap=ap)


def build(reps=1, debug_stage=None):
    nc = bacc.Bacc(None, target_bir_lowering=False, debug=False, num_devices=NCORES)
    xt_in = nc.dram_tensor("xt", [DM, S], BF16, kind="ExternalInput").ap()
    wqkv_in = nc.dram_tensor("wqkv", [L, DM, 384], BF16, kind="ExternalInput").ap()
    wo_in = nc.dram_tensor("wo", [L, QC, DM], BF16, kind="ExternalInput").ap()
    wg_in = nc.dram_tensor("wg", [L, DM, FFS], BF16, kind="ExternalInput").ap()
    wu_in = nc.dram_tensor("wu", [L, DM, FFS], BF16, kind="ExternalInput").ap()
    wd_in = nc.dram_tensor("wd", [L, FFS, DM], BF16, kind="ExternalInput").ap()
    cos_in = nc.dram_tensor("cosr", [128, S], BF16, kind="ExternalInput").ap()
    sin_in = nc.dram_tensor("sinr", [128, S], BF16, kind="ExternalInput").ap()
    mask_in = nc.dram_tensor("mask", [2, 128, 256], F32, kind="ExternalInput").ap()
    y_out = nc.dram_tensor("y", [DM, S], F32, kind="ExternalOutput").ap()

    with tile.TileContext(nc) as tc, ExitStack() as ctx:
        const = ctx.enter_context(tc.tile_pool(name="const", bufs=1))
        xtp = ctx.enter_context(tc.tile_pool(name="xtp", bufs=1))
        sq = ctx.enter_context(tc.tile_pool(name="sq", bufs=2))
        rp = ctx.enter_context(tc.tile_pool(name="rp", bufs=4))
        wqp = ctx.enter_context(tc.tile_pool(name="wqp", bufs=1))
        wob = ctx.enter_context(tc.tile_pool(name="wob", bufs=1))
        wbig = ctx.enter_context(tc.tile_pool(name="wbig", bufs=3))
        wdp = ctx.enter_context(tc.tile_pool(name="wdp", bufs=2))
        qksb = ctx.enter_context(tc.tile_pool(name="qksb", bufs=1))
        ropet = ctx.enter_context(tc.tile_pool(name="ropet", bufs=1))
        vap = ctx.enter_context(tc.tile_pool(name="vap", bufs=1))
        expp = ctx.enter_context(tc.tile_pool(name="expp", bufs=4))
        smp = ctx.enter_context(tc.tile_pool(name="smp", bufs=2))
        stkp = ctx.enter_context(tc.tile_pool(name="stkp", bufs=1))
        smallp = ctx.enter_context(tc.tile_pool(name="smallp", bufs=2))
        evp = ctx.enter_context(tc.tile_pool(name="evp", bufs=2))
        arp = ctx.enter_context(tc.tile_pool(name="arp", bufs=2))
        actp = ctx.enter_context(tc.tile_pool(name="actp", bufs=1))

        ccp = ctx.enter_context(tc.tile_pool(name="ccp", bufs=4, space="DRAM"))

        # ---- persistent constants ----
        onesb = const.tile([128, 1], BF16)
        nc.vector.memset(onesb[:], 1.0)
        ones_row = const.tile([1, 128], BF16)
        nc.vector.memset(ones_row[:], 1.0)
        id64 = const.tile([64, 64], BF16)
        masks.make_identity(nc, id64[:])
        cosr = const.tile([128, S], BF16)
        nc.sync.dma_start(cosr[:], cos_in[:])
        sinr = const.tile([128, S], BF16)
        nc.sync.dma_start(sinr[:], sin_in[:])
        mask0 = const.tile([128, 256], F32)
        nc.sync.dma_start(mask0[:], mask_in[0, :, :])
        mask1 = const.tile([128, 256], F32)
        nc.sync.dma_start(mask1[:], mask_in[1, :, :])
        epsb = const.tile([1, 1], F32)
        nc.vector.memset(epsb[:], EPS)

        # residual stream (bf16), resident
        xt = [xtp.tile([128, S], BF16, tag=f"xt{k}", name=f"xt{k}") for k in range(KT)]

        def load_x():
            # on the gpsimd queue: SP/HWDGE stays free for weight prefetch
            for k in range(KT):
                nc.gpsimd.dma_start(xt[k][:], xt_in[k * 128:(k + 1) * 128, :])

        def resid_add(cc_out_st, st):
            sl = slice(st * 512, (st + 1) * 512)
            ar = arp.tile([128, KT * 512], BF16, tag="ar", bufs=1)
            nc.sync.dma_start(
                ar[:], _ap3(cc_out_st[:], 0, [[512, 128], [128 * 512, KT], [1, 512]]))
            for k in range(KT):
                nc.vector.tensor_add(xt[k][:, sl], xt[k][:, sl],
                                     ar[:, k * 512:(k + 1) * 512])

        def rms_st(st):
            """rb = bcast(rsqrt(mean_dm(x^2)+eps)), [128,512] bf16."""
            sl = slice(st * 512, (st + 1) * 512)
            ctx_rms = ExitStack()
            ps_rms = ctx_rms.enter_context(
                tc.tile_pool(name="ps_rms", bufs=2, space="PSUM"))
            ssum = ps_rms.tile([1, 512], F32, tag="rmsps", name="ssum")
            for k in range(KT):
                xsq = sq.tile([128, 512], BF16, tag="xsq")
                nc.scalar.activation(xsq[:], xt[k][:, sl], AF.Square)
                nc.tensor.matmul(ssum[:], onesb[:], xsq[:],
                                 start=(k == 0), stop=(k == KT - 1))
            rs = smallp.tile([1, 512], F32, tag="rs")
            nc.scalar.activation(rs[:], ssum[:], AF.Sqrt,
                                 bias=epsb[:], scale=1.0 / DM)
            rr = smallp.tile([1, 512], BF16, tag="rr")
            with nc.allow_low_precision(reason="bf16 rsqrt broadcast is plenty"):
                nc.vector.reciprocal(rr[:], rs[:])
            rbp = ps_rms.tile([128, 512], F32, tag="rmsps", name="rbp")
            nc.tensor.matmul(rbp[:], ones_row[:], rr[:], start=True, stop=True)
            rb = rp.tile([128, 512], BF16, tag="rb", name=f"rb{st}")
            nc.vector.tensor_copy(rb[:], rbp[:])
            ctx_rms.close()
            return rb

        def rope(t_in, nrows, t_out):
            """t_out (bf16) = t_in * cos + rowswap(t_in) * sin_signed."""
            rot = ropet.tile([128, S], BF16, tag="rot")
            for h0 in range(0, nrows, 64):
                nc.sync.dma_start(rot[h0:h0 + 32, :], t_in[h0 + 32:h0 + 64, :])
                nc.sync.dma_start(rot[h0 + 32:h0 + 64, :], t_in[h0:h0 + 32, :])
            t1 = ropet.tile([128, S], BF16, tag="t1")
            nc.vector.tensor_tensor(t1[0:nrows, :], t_in[0:nrows, :],
                                    cosr[0:nrows, :], AluOpType.mult)
            nc.vector.tensor_tensor(rot[0:nrows, :], rot[0:nrows, :],
                                    sinr[0:nrows, :], AluOpType.mult)
            nc.vector.tensor_add(t_out[0:nrows, :], t1[0:nrows, :], rot[0:nrows, :])

        def attn_block(l, pending):
            # whole-layer qkv weights in one DMA
            wq_big = wqp.tile([128, KT * 384], BF16, tag="wq_big")
            nc.sync.dma_start(
                wq_big[:], _ap3(wqkv_in, l * DM * 384,
                                [[384, 128], [128 * 384, KT], [1, 384]]))

            q01s = qksb.tile([128, S], BF16, tag="q01s", name="q01s")
            q23s = qksb.tile([128, S], BF16, tag="q23s", name="q23s")
            ks = qksb.tile([64, S], BF16, tag="ks", name="ks")
            vs = qksb.tile([64, S], BF16, tag="vs", name="vs")
            for st in range(2):
                if pending is not None:
                    resid_add(pending[st], st)
                rb = rms_st(st)
                sl = slice(st * 512, (st + 1) * 512)
                ctx_qkv = ExitStack()
                ps_qkv = ctx_qkv.enter_context(
                    tc.tile_pool(name="ps_qkv", bufs=1, space="PSUM"))
                pq0 = ps_qkv.tile([128, 512], F32, tag="pq0", name="pq0")
                pq1 = ps_qkv.tile([128, 512], F32, tag="pq1", name="pq1")
                pkv = ps_qkv.tile([128, 512], F32, tag="pkv", name="pkv")
                for k in range(KT):
                    w0 = k * 384
                    st_, sp_ = (k == 0), (k == KT - 1)
                    nc.tensor.matmul(pq0[:], wq_big[:, w0:w0 + 128],
                                     xt[k][:, sl], start=st_, stop=sp_)
                    nc.tensor.matmul(pq1[:], wq_big[:, w0 + 128:w0 + 256],
                                     xt[k][:, sl], start=st_, stop=sp_)
                    nc.tensor.matmul(pkv[:], wq_big[:, w0 + 256:w0 + 384],
                                     xt[k][:, sl], start=st_, stop=sp_)
                nc.vector.tensor_tensor(q01s[:, sl], pq0[:], rb[:], AluOpType.mult)
                nc.vector.tensor_tensor(q23s[:, sl], pq1[:], rb[:], AluOpType.mult)
                nc.vector.tensor_tensor(ks[0:64, sl], pkv[0:64, :], rb[0:64, :],
                                        AluOpType.mult)
                nc.vector.tensor_tensor(vs[0:64, sl], pkv[64:128, :], rb[64:128, :],
                                        AluOpType.mult)
                ctx_qkv.close()

            # RoPE into bf16 attention operands
            q01 = qksb.tile([128, S], BF16, tag="q01", name="q01")
            q23 = qksb.tile([128, S], BF16, tag="q23", name="q23")
            kt2 = qksb.tile([128, S], BF16, tag="kt2", name="kt2")
            rope(q01s, 128, q01)
            rope(q23s, 128, q23)
            rope(ks, 64, kt2)
            nc.sync.dma_start(kt2[64:128, :], kt2[0:64, :])  # duplicate kv head

            # v -> natural [s, d] via PE transpose, plus ones column for sums
            ctx_vt = ExitStack()
            ps_vt = ctx_vt.enter_context(
                tc.tile_pool(name="ps_vt", bufs=2, space="PSUM"))
            va = []
            for sj in range(8):
                tp = ps_vt.tile([128, 64], BF16, tag="vtp")
                nc.tensor.transpose(tp[:], vs[0:64, sj * 128:(sj + 1) * 128], id64[:])
                v = vap.tile([128, 66], BF16, tag=f"va{sj}", name=f"va{sj}")
                nc.vector.tensor_copy(v[:, 0:64], tp[:])
                nc.vector.tensor_copy(v[:, 64:65], onesb[:])
                va.append(v)
            ctx_vt.close()

            # attention: causal over 128-wide j blocks, 256-wide i blocks.
            # heads interleaved so row-tiled (64-row) score matmuls pack.
            stk0 = stkp.tile([128, S], BF16, tag="stk0", name="stk0")
            stk1 = stkp.tile([128, S], BF16, tag="stk1", name="stk1")
            ctx_att = ExitStack()
            ps_sc = ctx_att.enter_context(
                tc.tile_pool(name="ps_sc", bufs=3, space="PSUM"))
            ps_at = ctx_att.enter_context(
                tc.tile_pool(name="ps_at", bufs=4, space="PSUM"))
            ps_wo = ctx_att.enter_context(
                tc.tile_pool(name="ps_wo", bufs=1, space="PSUM"))
            wo_big = wob.tile([128, 2 * DM], BF16, tag="wo_big")
            nc.sync.dma_start(
                wo_big[:], _ap3(wo_in, l * QC * DM,
                                [[DM, 128], [128 * DM, 2], [1, DM]]))
            cc = []

            def attn_it(it):
                isl = slice(it * 256, (it + 1) * 256)
                jmax = 2 * it + 2
                aps = [ps_at.tile([66, 256], F32, tag="aps", name=f"aps{h}")
                       for h in range(4)]
                for j in range(jmax):
                    jsl = slice(j * 128, (j + 1) * 128)
                    for h in range(4):
                        half = slice(64 * (h % 2), 64 * (h % 2) + 64)
                        qt = (q01, q23)[h // 2]
                        sps = ps_sc.tile([128, 256], F32, tag="sps")
                        nc.tensor.matmul(sps[:], kt2[half, jsl], qt[half, isl],
                                         start=True, stop=True)
                        e = expp.tile([128, 256], BF16, tag="e")
                        if j >= 2 * it:
                            sm = smp.tile([128, 256], F32, tag="sm")
                            nc.vector.tensor_add(sm[:], sps[:],
                                                 (mask0, mask1)[j - 2 * it][:])
                            nc.scalar.activation(e[:], sm[:], AF.Exp)
                        else:
                            nc.scalar.activation(e[:], sps[:], AF.Exp)
                        nc.tensor.matmul(aps[h][0:65, :], va[j][:, 0:65], e[:],
                                         start=(j == 0), stop=(j == jmax - 1))
                for h in range(4):
                    srow = smallp.tile([1, 256], F32, tag="srow")
                    nc.vector.tensor_copy(srow[:], aps[h][64:65, :])
                    rinv = smallp.tile([1, 256], BF16, tag="rinv")
                    with nc.allow_low_precision(reason="bf16 softmax denom"):
                        nc.vector.reciprocal(rinv[:], srow[:])
                    rbp = ps_sc.tile([64, 256], F32, tag="sps", name="rbp")
                    nc.tensor.matmul(rbp[:], ones_row[0:1, 0:64], rinv[:],
                                     start=True, stop=True)
                    rbs = smallp.tile([64, 256], BF16, tag="rbs")
                    nc.vector.tensor_copy(rbs[:], rbp[:])
                    stk = (stk0, stk1)[h // 2]
                    rows = slice(64 * (h % 2), 64 * (h % 2) + 64)
                    nc.vector.tensor_tensor(stk[rows, isl], aps[h][0:64, :],
                                            rbs[:], AluOpType.mult)

            def wo_st(st):
                sl = slice(st * 512, (st + 1) * 512)
                cc_in = ccp.tile([DM, 512], BF16, tag="cc_in")
                cc_out = ccp.tile([DM, 512], BF16, tag="cc_out", name=f"cco_a{l}{st}",
                                  addr_space="Shared")
                pr = evp.tile([128, KT * 512], BF16, tag="pr", bufs=1)
                for dmm in range(KT):
                    wops = ps_wo.tile([128, 512], F32, tag="wops")
                    nc.tensor.matmul(wops[:], wo_big[:, dmm * 128:(dmm + 1) * 128],
                                     stk0[:, sl], start=True, stop=False)
                    nc.tensor.matmul(wops[:],
                                     wo_big[:, DM + dmm * 128:DM + (dmm + 1) * 128],
                                     stk1[:, sl], start=False, stop=True)
                    nc.vector.tensor_copy(pr[:, dmm * 512:(dmm + 1) * 512], wops[:])
                nc.sync.dma_start(
                    _ap3(cc_in[:], 0, [[512, 128], [128 * 512, KT], [1, 512]]),
                    pr[:])
                nc.gpsimd.collective_compute(
                    "AllReduce", AluOpType.add,
                    replica_groups=[list(range(NCORES))],
                    ins=[cc_in[:].opt()], outs=[cc_out[:].opt()])
                cc.append(cc_out)

            attn_it(0)
            attn_it(1)
            wo_st(0)
            attn_it(2)
            attn_it(3)
            wo_st(1)
            ctx_att.close()
            return cc

        def mlp_st(l, st, last, gt, rb):
            sl = slice(st * 512, (st + 1) * 512)
            ctx_mlp = ExitStack()
            ps_mlp = ctx_mlp.enter_context(
                tc.tile_pool(name="ps_mlp", bufs=2, space="PSUM"))
            for name, w_in in (("g", wg_in), ("u", wu_in)):
                for fp in range(2):
                    wh = []
                    for hh in range(2):
                        wt = wbig.tile([128, 8 * 512], BF16, tag="wbig",
                                       name=f"w{name}{fp}{hh}")
                        nc.sync.dma_start(
                            wt[:], _ap3(w_in, l * DM * FFS + fp * 512 + hh * 8 * 128 * FFS,
                                        [[FFS, 128], [128 * FFS, 8], [1, 512]]))
                        wh.append(wt)
                    ps = [ps_mlp.tile([128, 512], F32, tag=f"mps{fm}", name=f"mps{fm}")
                          for fm in range(4)]
                    for k in range(KT):
                        wt = wh[k // 8]
                        c0 = (k % 8) * 512
                        for fm in range(4):
                            nc.tensor.matmul(
                                ps[fm][:],
                                wt[:, c0 + fm * 128:c0 + (fm + 1) * 128],
                                xt[k][:, sl], start=(k == 0), stop=(k == KT - 1))
                    for fm in range(4):
                        fi = fp * 4 + fm
                        # scale by rsqrt in psum, then activate / multiply
                        nc.vector.tensor_tensor(ps[fm][:], ps[fm][:], rb[:],
                                                AluOpType.mult)
                        if name == "g":
                            nc.scalar.activation(gt[fi][:, sl], ps[fm][:], AF.Silu)
                        else:
                            nc.vector.tensor_tensor(gt[fi][:, sl], ps[fm][:],
                                                    gt[fi][:, sl], AluOpType.mult)
            if not last:
                cc_in = ccp.tile([DM, 512], BF16, tag="cc_in")
                cc_out = ccp.tile([DM, 512], BF16, tag="cc_out", name=f"cco_m{l}{st}",
                                  addr_space="Shared")
                prb = evp.tile([128, KT * 512], BF16, tag="pr", bufs=1, name=f"prb{l}{st}")
            for dp in range(4):
                wdt = wdp.tile([128, 8 * 512], BF16, tag="wdt")
                nc.sync.dma_start(
                    wdt[:], _ap3(wd_in, l * FFS * DM + dp * 512,
                                 [[DM, 128], [128 * DM, 8], [1, 512]]))
                dps = [ps_mlp.tile([128, 512], F32, tag=f"mps{dmm}", name=f"mpsd{dmm}")
                       for dmm in range(4)]
                for fk in range(8):
                    for dmm in range(4):
                        nc.tensor.matmul(
                            dps[dmm][:],
                            wdt[:, fk * 512 + dmm * 128:fk * 512 + (dmm + 1) * 128],
                            gt[fk][:, sl], start=(fk == 0), stop=(fk == 7))
                if last:
                    prd = evp.tile([128, 4 * 512], F32, tag="prd", bufs=1,
                                   name=f"prd{dp}")
                for dmm in range(4):
                    kk = dp * 4 + dmm
                    if last:
                        nc.vector.scalar_tensor_tensor(
                            prd[:, dmm * 512:(dmm + 1) * 512], xt[kk][:, sl],
                            1.0 / NCORES, dps[dmm][:],
                            AluOpType.mult, AluOpType.add)
                    else:
                        nc.vector.tensor_copy(prb[:, kk * 512:(kk + 1) * 512],
                                              dps[dmm][:])
                if last:
                    nc.sync.dma_start(
                        _ap3(y_out, dp * 512 * S + st * 512,
                             [[S, 128], [128 * S, 4], [1, 512]]),
                        prd[:])
            ctx_mlp.close()
            if last:
                return None
            nc.sync.dma_start(
                _ap3(cc_in[:], 0, [[512, 128], [128 * 512, KT], [1, 512]]),
                prb[:])
            nc.gpsimd.collective_compute(
                "AllReduce", AluOpType.add,
                replica_groups=[list(range(NCORES))],
                ins=[cc_in[:].opt()], outs=[cc_out[:].opt()])
            return cc_out

        def mlp_block(l, pending, last):
            gt = [actp.tile([128, S], BF16, tag=f"g{fi}", name=f"g{fi}")
                  for fi in range(8)]
            cc = []
            for st in range(2):
                resid_add(pending[st], st)
                rb = rms_st(st)
                cc.append(mlp_st(l, st, last, gt, rb))
            return None if last else cc

        def dump_x():
            for k in range(KT):
                pr = evp.tile([128, S], F32, tag="dbg")
                nc.vector.tensor_copy(pr[:], xt[k][:])
                nc.sync.dma_start(y_out[k * 128:(k + 1) * 128, :], pr[:])

        for _ in range(reps):
            load_x()
            pending = None
            done = False
            for l in range(L):
                pending = attn_block(l, pending)
                if debug_stage == f"attn{l}":
                    for st in range(2):
                        resid_add(pending[st], st)
                    dump_x()
                    done = True
                    break
                last = (l == L - 1 and debug_stage is None)
                pending = mlp_block(l, pending, last)
                if debug_stage == f"mlp{l}":
                    for st in range(2):
                        resid_add(pending[st], st)
                    dump_x()
                    done = True
                    break
            if done:
                break

    nc.compile()
    return nc


def make_inputs(input_ids, embed, wq, wk, wv, wo, wgate, wup, wdown, ln1, ln2):
    """host-side prep: embedding gather, shard + fold norm gains/scale into weights."""
    f32 = np.float32
    bf = ml_dtypes.bfloat16
    x = np.asarray(embed, f32)[np.asarray(input_ids)[0]]      # (S, DM)
    xt = np.ascontiguousarray(x.T).astype(bf)                 # (DM, S) bf16

    inv_freq = 1.0 / (THETA ** (np.arange(0, D, 2, dtype=f32) / D))
    freqs = np.arange(S, dtype=f32)[:, None] * inv_freq[None, :]    # (S, 32)
    emb = np.concatenate([freqs, freqs], axis=1)                    # (S, D)
    cosT = np.cos(emb).T.astype(f32)                                # (D, S)
    sinT = np.sin(emb).T.astype(f32)
    sinT_signed = sinT.copy()
    sinT_signed[: D // 2] *= -1.0
    cos_rep = np.concatenate([cosT, cosT], axis=0).astype(bf)       # (128, S) bf16
    sin_rep = np.concatenate([sinT_signed, sinT_signed], axis=0).astype(bf)

    mask = np.zeros((2, 128, 256), f32)
    jj = np.arange(128)[:, None]
    ii = np.arange(256)[None, :]
    for o in range(2):
        mask[o] = np.where(128 * o + jj <= ii, 0.0, NEG)

    scale_q = 1.0 / math.sqrt(D)
    in_maps = []
    for c in range(NCORES):
        wqkv = np.empty((L, DM, 384), f32)
        wo_c = np.empty((L, QC, DM), f32)
        wg_c = np.empty((L, DM, FFS), f32)
        wu_c = np.empty((L, DM, FFS), f32)
        wd_c = np.empty((L, FFS, DM), f32)
        for l in range(L):
            g1 = np.asarray(ln1[l], f32)[:, None]
            g2 = np.asarray(ln2[l], f32)[:, None]
            wqkv[l, :, :QC] = np.asarray(wq[l], f32)[:, c * QC:(c + 1) * QC] * g1 * scale_q
            wqkv[l, :, QC:QC + D] = np.asarray(wk[l], f32)[:, c * D:(c + 1) * D] * g1
            wqkv[l, :, QC + D:] = np.asarray(wv[l], f32)[:, c * D:(c + 1) * D] * g1
            wo_c[l] = np.asarray(wo[l], f32)[c * QC:(c + 1) * QC, :]
            wg_c[l] = np.asarray(wgate[l], f32)[:, c * FFS:(c + 1) * FFS] * g2
            wu_c[l] = np.asarray(wup[l], f32)[:, c * FFS:(c + 1) * FFS] * g2
            wd_c[l] = np.asarray(wdown[l], f32)[c * FFS:(c + 1) * FFS, :]
        in_maps.append({
            "xt": xt, "wqkv": wqkv.astype(bf),
            "wo": wo_c.astype(bf), "wg": wg_c.astype(bf), "wu": wu_c.astype(bf),
            "wd": wd_c.astype(bf), "cosr": cos_rep, "sinr": sin_rep, "mask": mask,
        })
    return in_maps


_NC_CACHE = {}


def kernel(**inputs) -> np.ndarray:
    if 1 not in _NC_CACHE:
        _NC_CACHE[1] = build(reps=1)
    nc = _NC_CACHE[1]
    in_maps = make_inputs(**inputs)
    res = run_bass_kernel_spmd(nc, in_maps, list(range(NCORES)))
    y = np.zeros((DM, S), np.float64)
    for c in range(NCORES):
        y += res.results[c]["y"].astype(np.float64)
    return np.ascontiguousarray(y.T.astype(np.float32)).reshape(B, S, DM)


# revision 3
# speedup vs baseline: 2.0341x; 2.0341x over previous
"""Llama trunk (2 layers, before final norm) on 8 trn2 cores — v2.

Megatron tensor-parallel over 8 cores (4 q heads + 1 kv head, 1024 of
8192 FF dims per core).  Residual x lives on-chip in bf16, transposed
([DM, S]); all matmuls contract over the partition axis in bf16.
RMSNorm's rsqrt is broadcast via a K=1 matmul and applied during psum
evictions; gains and 1/sqrt(D) fold into weights on the host.  v comes
out of the combined k|v matmul and is PE-transposed to natural layout.
AllReduces run in bf16, split by sequence halves, issued early to
overlap attention/MLP compute.  All weight / collective staging DMAs
are batched into single multi-dim-AP transfers (per-dma_start dispatch
cost dominates otherwise).  Final down-proj partials reduce on host.
"""
import math
from contextlib import ExitStack

import ml_dtypes
import numpy as np

import concourse.bass as bass
import concourse.tile as tile
from concourse import bacc, masks, mybir
from concourse.alu_op_type import AluOpType
from concourse.bass_utils import run_bass_kernel_spmd

L, H, KVH, D = 2, 32, 8, 64
DM, FF = 2048, 8192
B, S = 1, 1024
EPS, THETA = 1e-5, 10000.0
NCORES = 8
QH = H // NCORES            # 4 q heads per core
QC = QH * D                 # 256 q cols per core
FFS = FF // NCORES          # 1024 ff dims per core
KT = DM // 128              # 16 contraction tiles over DM
NEG = -1.0e30

F32 = mybir.dt.float32
F32R = mybir.dt.float32r
BF16 = mybir.dt.bfloat16
AF = mybir.ActivationFunctionType


def _ap3(base_ap, offset_elems, ap):
    return bass.AP(tensor=base_ap.tensor, offset=base_ap.offset + offset_elems,
                   ap=ap)


PHASES = []


def _mark(nc, label):
    n = nc.get_next_instruction_name()  # consumes one name; fine for profiling
    PHASES.append((label, int(n.split("-")[1])))


def build(reps=1, debug_stage=None, mark=False, no_ar=False):
    PHASES.clear()
    nc = bacc.Bacc(None, target_bir_lowering=False, debug=False, num_devices=NCORES)
    mk = (lambda lbl: _mark(nc, lbl)) if mark else (lambda lbl: None)
    xt_in = nc.dram_tensor("xt", [DM, S], BF16, kind="ExternalInput").ap()
    wqkv_in = nc.dram_tensor("wqkv", [L, DM, 384], BF16, kind="ExternalInput").ap()
    wo_in = nc.dram_tensor("wo", [L, QC, DM], BF16, kind="ExternalInput").ap()
    wg_in = nc.dram_tensor("wg", [L, DM, FFS], BF16, kind="ExternalInput").ap()
    wu_in = nc.dram_tensor("wu", [L, DM, FFS], BF16, kind="ExternalInput").ap()
    wd_in = nc.dram_tensor("wd", [L, FFS, DM], BF16, kind="ExternalInput").ap()
    cos_in = nc.dram_tensor("cosr", [128, S], BF16, kind="ExternalInput").ap()
    sin_in = nc.dram_tensor("sinr", [128, S], BF16, kind="ExternalInput").ap()
    mask_in = nc.dram_tensor("mask", [2, 128, 256], F32, kind="ExternalInput").ap()
    y_out = nc.dram_tensor("y", [DM, S], F32, kind="ExternalOutput").ap()

    with tile.TileContext(nc) as tc, ExitStack() as ctx:
        const = ctx.enter_context(tc.tile_pool(name="const", bufs=1))
        xtp = ctx.enter_context(tc.tile_pool(name="xtp", bufs=1))
        sq = ctx.enter_context(tc.tile_pool(name="sq", bufs=2))
        rp = ctx.enter_context(tc.tile_pool(name="rp", bufs=4))
        wqp = ctx.enter_context(tc.tile_pool(name="wqp", bufs=1))
        wob = ctx.enter_context(tc.tile_pool(name="wob", bufs=1))
        wbig = ctx.enter_context(tc.tile_pool(name="wbig", bufs=3))
        wdp = ctx.enter_context(tc.tile_pool(name="wdp", bufs=2))
        qksb = ctx.enter_context(tc.tile_pool(name="qksb", bufs=1))
        ropet = ctx.enter_context(tc.tile_pool(name="ropet", bufs=1))
        vap = ctx.enter_context(tc.tile_pool(name="vap", bufs=1))
        expp = ctx.enter_context(tc.tile_pool(name="expp", bufs=4))
        smp = ctx.enter_context(tc.tile_pool(name="smp", bufs=2))
        stkp = ctx.enter_context(tc.tile_pool(name="stkp", bufs=1))
        smallp = ctx.enter_context(tc.tile_pool(name="smallp", bufs=2))
        evp = ctx.enter_context(tc.tile_pool(name="evp", bufs=2))
        arp = ctx.enter_context(tc.tile_pool(name="arp", bufs=2))
        actp = ctx.enter_context(tc.tile_pool(name="actp", bufs=1))

        ccp = ctx.enter_context(tc.tile_pool(name="ccp", bufs=4, space="DRAM"))

        # ---- persistent constants ----
        onesb = const.tile([128, 1], BF16)
        nc.vector.memset(onesb[:], 1.0)
        ones_row = const.tile([1, 128], BF16)
        nc.vector.memset(ones_row[:], 1.0)
        id64 = const.tile([64, 64], BF16)
        masks.make_identity(nc, id64[:])
        p128 = const.tile([128, 128], BF16)
        nc.gpsimd.memset(p128[:], 0.0)
        for g in range(2):
            for half in range(2):
                masks.make_identity(
                    nc, p128[g * 64 + half * 32:g * 64 + half * 32 + 32,
                             g * 64 + (1 - half) * 32:g * 64 + (1 - half) * 32 + 32],
                    nomemset=True)
        cosr = const.tile([128, S], BF16)
        nc.sync.dma_start(cosr[:], cos_in[:])
        sinr = const.tile([128, S], BF16)
        nc.sync.dma_start(sinr[:], sin_in[:])
        mask0 = const.tile([128, 256], F32)
        nc.sync.dma_start(mask0[:], mask_in[0, :, :])
        mask1 = const.tile([128, 256], F32)
        nc.sync.dma_start(mask1[:], mask_in[1, :, :])
        epsb = const.tile([1, 1], F32)
        nc.vector.memset(epsb[:], EPS)

        # residual stream (bf16), resident
        xt = [xtp.tile([128, S], BF16, tag=f"xt{k}", name=f"xt{k}") for k in range(KT)]

        def load_x():
            # split across the SWDGE (Pool) and HWDGE (SP) queues so the
            # last tile lands sooner; consumers pipeline per k-tile.
            for k in range(KT):
                eng = nc.gpsimd if k % 2 else nc.sync
                eng.dma_start(xt[k][:], xt_in[k * 128:(k + 1) * 128, :])

        def resid_add(cc_out_st, st):
            sl = slice(st * 512, (st + 1) * 512)
            ar = arp.tile([128, KT * 512], BF16, tag="ar", bufs=1)
            for c in range(4):
                nc.sync.dma_start(
                    ar[:, c * 4 * 512:(c + 1) * 4 * 512],
                    _ap3(cc_out_st[:], c * 4 * 128 * 512,
                         [[512, 128], [128 * 512, 4], [1, 512]]))
            for k in range(KT):
                nc.vector.tensor_add(xt[k][:, sl], xt[k][:, sl],
                                     ar[:, k * 512:(k + 1) * 512])

        def rms_st(st):
            """rb = bcast(rsqrt(mean_dm(x^2)+eps)), [128,512] bf16."""
            sl = slice(st * 512, (st + 1) * 512)
            ctx_rms = ExitStack()
            ps_rms = ctx_rms.enter_context(
                tc.tile_pool(name="ps_rms", bufs=1, space="PSUM"))
            ssum = ps_rms.tile([1, 512], F32, tag="rmsps", name="ssum")
            for k in range(KT):
                xsq = sq.tile([128, 512], BF16, tag="xsq")
                nc.scalar.activation(xsq[:], xt[k][:, sl], AF.Square)
                nc.tensor.matmul(ssum[:], onesb[:], xsq[:],
                                 start=(k == 0), stop=(k == KT - 1))
            rs = smallp.tile([1, 512], F32, tag="rs")
            nc.scalar.activation(rs[:], ssum[:], AF.Sqrt,
                                 bias=epsb[:], scale=1.0 / DM)
            rr = smallp.tile([1, 512], BF16, tag="rr")
            with nc.allow_low_precision(reason="bf16 rsqrt broadcast is plenty"):
                nc.vector.reciprocal(rr[:], rs[:])
            rbp = ps_rms.tile([128, 512], F32, tag="rmsps", name="rbp")
            nc.tensor.matmul(rbp[:], ones_row[:], rr[:], start=True, stop=True)
            rb = rp.tile([128, 512], BF16, tag="rb", name=f"rb{st}")
            nc.vector.tensor_copy(rb[:], rbp[:])
            ctx_rms.close()
            return rb

        def rope(t_in, nrows, t_out, csl, ps_pool):
            """t_out[:, csl] (bf16) = (t_in*cos + rowswap(t_in)*sin_signed)[:, csl]."""
            rotp = ps_pool.tile([128, 512], F32, tag="rotp", bufs=1, name="rotp")
            nc.tensor.matmul(rotp[0:nrows, :], p128[0:nrows, 0:nrows],
                             t_in[0:nrows, csl], start=True, stop=True)
            t1 = ropet.tile([128, 512], BF16, tag="t1")
            nc.vector.tensor_tensor(t1[0:nrows, :], t_in[0:nrows, csl],
                                    cosr[0:nrows, csl], AluOpType.mult)
            rot = ropet.tile([128, 512], BF16, tag="rot")
            nc.vector.tensor_tensor(rot[0:nrows, :], rotp[0:nrows, :],
                                    sinr[0:nrows, csl], AluOpType.mult)
            nc.vector.tensor_add(t_out[0:nrows, csl], t1[0:nrows, :],
                                 rot[0:nrows, :])

        def attn_block(l, pending):
            # whole-layer qkv + wo weights, one DMA each (early prefetch)
            wq_big = wqp.tile([128, KT * 384], BF16, tag="wq_big")
            nc.sync.dma_start(
                wq_big[:], _ap3(wqkv_in, l * DM * 384,
                                [[384, 128], [128 * 384, KT], [1, 384]]))
            wo_big = wob.tile([128, 2 * DM], BF16, tag="wo_big")
            nc.sync.dma_start(
                wo_big[:], _ap3(wo_in, l * QC * DM,
                                [[DM, 128], [128 * DM, 2], [1, DM]]))

            q01s = qksb.tile([128, S], BF16, tag="q01s", name="q01s")
            q23s = qksb.tile([128, S], BF16, tag="q23s", name="q23s")
            ks = qksb.tile([64, S], BF16, tag="ks", name="ks")
            vs = qksb.tile([64, S], BF16, tag="vs", name="vs")
            q01 = qksb.tile([128, S], BF16, tag="q01", name="q01")
            q23 = qksb.tile([128, S], BF16, tag="q23", name="q23")
            kt2 = qksb.tile([128, S], BF16, tag="kt2", name="kt2")
            stk0 = stkp.tile([128, S], BF16, tag="stk0", name="stk0")
            stk1 = stkp.tile([128, S], BF16, tag="stk1", name="stk1")
            va = [None] * 8

            ctx_att = ExitStack()
            ps_sc = ctx_att.enter_context(
                tc.tile_pool(name="ps_sc", bufs=3, space="PSUM"))
            ps_at = ctx_att.enter_context(
                tc.tile_pool(name="ps_at", bufs=2, space="PSUM"))
            cc = []

            def qkv_st(st):
                if pending is not None:
                    resid_add(pending[st], st)
                rb = rms_st(st)
                sl = slice(st * 512, (st + 1) * 512)
                ctx_qkv = ExitStack()
                ps_qkv = ctx_qkv.enter_context(
                    tc.tile_pool(name="ps_qkv", bufs=1, space="PSUM"))
                pq0 = ps_qkv.tile([128, 512], F32, tag="pq0", name="pq0")
                pq1 = ps_qkv.tile([128, 512], F32, tag="pq1", name="pq1")
                pkv = ps_qkv.tile([128, 512], F32, tag="pkv", name="pkv")
                for k in range(KT):
                    w0 = k * 384
                    st_, sp_ = (k == 0), (k == KT - 1)
                    nc.tensor.matmul(pq0[:], wq_big[:, w0:w0 + 128],
                                     xt[k][:, sl], start=st_, stop=sp_)
                    nc.tensor.matmul(pq1[:], wq_big[:, w0 + 128:w0 + 256],
                                     xt[k][:, sl], start=st_, stop=sp_)
                    nc.tensor.matmul(pkv[:], wq_big[:, w0 + 256:w0 + 384],
                                     xt[k][:, sl], start=st_, stop=sp_)
                nc.vector.tensor_tensor(q01s[:, sl], pq0[:], rb[:], AluOpType.mult)
                nc.vector.tensor_tensor(q23s[:, sl], pq1[:], rb[:], AluOpType.mult)
                nc.vector.tensor_tensor(ks[0:64, sl], pkv[0:64, :], rb[0:64, :],
                                        AluOpType.mult)
                nc.vector.tensor_tensor(vs[0:64, sl], pkv[64:128, :], rb[64:128, :],
                                        AluOpType.mult)
                ctx_qkv.close()

            def prep_half(st):
                """RoPE + v-transpose for this half's columns only."""
                csl = slice(st * 512, (st + 1) * 512)
                ctx_vt = ExitStack()
                ps_vt = ctx_vt.enter_context(
                    tc.tile_pool(name="ps_vt", bufs=2, space="PSUM"))
                rope(q01s, 128, q01, csl, ps_vt)
                rope(q23s, 128, q23, csl, ps_vt)
                rope(ks, 64, kt2, csl, ps_vt)
                nc.sync.dma_start(kt2[64:128, csl], kt2[0:64, csl])
                for sj in range(st * 4, st * 4 + 4):
                    tp = ps_vt.tile([128, 64], BF16, tag="vtp")
                    nc.tensor.transpose(tp[:], vs[0:64, sj * 128:(sj + 1) * 128],
                                        id64[:])
                    v = vap.tile([128, 66], BF16, tag=f"va{sj}", name=f"va{sj}")
                    nc.vector.tensor_copy(v[:, 0:64], tp[:])
                    nc.vector.tensor_copy(v[:, 64:65], onesb[:])
                    va[sj] = v
                ctx_vt.close()

            def attn_it(it):
                isl = slice(it * 256, (it + 1) * 256)
                jmax = 2 * it + 2
                for hp in range(2):
                    qt = (q01, q23)[hp]
                    aps = [ps_at.tile([66, 256], F32, tag="aps", name=f"aps{i}")
                           for i in range(2)]
                    for j in range(jmax):
                        jsl = slice(j * 128, (j + 1) * 128)
                        for i01 in range(2):
                            half = slice(64 * i01, 64 * i01 + 64)
                            sps = ps_sc.tile([128, 256], F32, tag="sps")
                            nc.tensor.matmul(sps[:], kt2[half, jsl], qt[half, isl],
                                             start=True, stop=True)
                            e = expp.tile([128, 256], BF16, tag="e")
                            if j >= 2 * it:
                                sm = smp.tile([128, 256], F32, tag="sm")
                                nc.vector.tensor_add(sm[:], sps[:],
                                                     (mask0, mask1)[j - 2 * it][:])
                                nc.scalar.activation(e[:], sm[:], AF.Exp)
                            else:
                                nc.scalar.activation(e[:], sps[:], AF.Exp)
                            nc.tensor.matmul(aps[i01][0:65, :], va[j][:, 0:65], e[:],
                                             start=(j == 0), stop=(j == jmax - 1))
                    # evacuate psum accumulators early, normalize from SBUF
                    ats = []
                    for i01 in range(2):
                        at = smallp.tile([66, 256], BF16, tag="atile", name=f"at{i01}")
                        nc.vector.tensor_copy(at[0:65, :], aps[i01][0:65, :])
                        ats.append(at)
                    stk = (stk0, stk1)[hp]
                    for i01 in range(2):
                        rinv = smallp.tile([1, 256], BF16, tag="rinv")
                        with nc.allow_low_precision(reason="bf16 softmax denom"):
                            nc.vector.reciprocal(rinv[:], ats[i01][64:65, :])
                        rbp = ps_sc.tile([64, 256], F32, tag="sps", name="rbp")
                        nc.tensor.matmul(rbp[:], ones_row[0:1, 0:64], rinv[:],
                                         start=True, stop=True)
                        rows = slice(64 * i01, 64 * i01 + 64)
                        nc.vector.tensor_tensor(stk[rows, isl], ats[i01][0:64, :],
                                                rbp[:], AluOpType.mult)

            def wo_st(st):
                sl = slice(st * 512, (st + 1) * 512)
                cc_in = ccp.tile([DM, 512], BF16, tag="cc_in")
                cc_out = ccp.tile([DM, 512], BF16, tag="cc_out", name=f"cco_a{l}{st}",
                                  addr_space="Shared")
                ctx_wo = ExitStack()
                ps_wo = ctx_wo.enter_context(
                    tc.tile_pool(name="ps_wo", bufs=1, space="PSUM"))
                pr = evp.tile([128, KT * 512], BF16, tag="pr", bufs=1)
                for dmm in range(KT):
                    wops = ps_wo.tile([128, 512], F32, tag="wops")
                    nc.tensor.matmul(wops[:], wo_big[:, dmm * 128:(dmm + 1) * 128],
                                     stk0[:, sl], start=True, stop=False)
                    nc.tensor.matmul(wops[:],
                                     wo_big[:, DM + dmm * 128:DM + (dmm + 1) * 128],
                                     stk1[:, sl], start=False, stop=True)
                    nc.vector.tensor_copy(pr[:, dmm * 512:(dmm + 1) * 512], wops[:])
                ctx_wo.close()
                tgt = cc_out if no_ar else cc_in
                nc.sync.dma_start(
                    _ap3(tgt[:], 0, [[512, 128], [128 * 512, KT], [1, 512]]),
                    pr[:])
                if not no_ar:
                    nc.gpsimd.collective_compute(
                        "AllReduce", AluOpType.add,
                        replica_groups=[list(range(NCORES))],
                        ins=[cc_in[:].opt()], outs=[cc_out[:].opt()])
                cc.append(cc_out)

            # st0 half first: it0/it1 only need j < 512, so they run while
            # the st1-half AllReduce from the previous block is in flight.
            mk(f"L{l}.qkv0")
            qkv_st(0)
            prep_half(0)
            mk(f"L{l}.it01")
            attn_it(0)
            attn_it(1)
            mk(f"L{l}.wo0")
            wo_st(0)
            mk(f"L{l}.qkv1")
            qkv_st(1)
            prep_half(1)
            mk(f"L{l}.it23")
            attn_it(2)
            attn_it(3)
            mk(f"L{l}.wo1")
            wo_st(1)
            ctx_att.close()
            return cc

        def mlp_st(l, st, last, gt, rb):
            sl = slice(st * 512, (st + 1) * 512)
            ctx_mlp = ExitStack()
            ps_mlp = ctx_mlp.enter_context(
                tc.tile_pool(name="ps_mlp", bufs=2, space="PSUM"))
            for name, w_in in (("g", wg_in), ("u", wu_in)):
                for fp in range(2):
                    wh = []
                    for hh in range(2):
                        wt = wbig.tile([128, 8 * 512], BF16, tag="wbig",
                                       name=f"w{name}{fp}{hh}")
                        nc.sync.dma_start(
                            wt[:], _ap3(w_in, l * DM * FFS + fp * 512 + hh * 8 * 128 * FFS,
                                        [[FFS, 128], [128 * FFS, 8], [1, 512]]))
                        wh.append(wt)
                    ps = [ps_mlp.tile([128, 512], F32, tag=f"mps{fm}", name=f"mps{fm}")
                          for fm in range(4)]
                    for k in range(KT):
                        wt = wh[k // 8]
                        c0 = (k % 8) * 512
                        for fm in range(4):
                            nc.tensor.matmul(
                                ps[fm][:],
                                wt[:, c0 + fm * 128:c0 + (fm + 1) * 128],
                                xt[k][:, sl], start=(k == 0), stop=(k == KT - 1))
                    for fm in range(4):
                        fi = fp * 4 + fm
                        # scale by rsqrt in psum, then activate / multiply
                        nc.vector.tensor_tensor(ps[fm][:], ps[fm][:], rb[:],
                                                AluOpType.mult)
                        if name == "g":
                            nc.scalar.activation(gt[fi][:, sl], ps[fm][:], AF.Silu)
                        else:
                            nc.vector.tensor_tensor(gt[fi][:, sl], ps[fm][:],
                                                    gt[fi][:, sl], AluOpType.mult)
            if not last:
                cc_in = ccp.tile([DM, 512], BF16, tag="cc_in")
                cc_out = ccp.tile([DM, 512], BF16, tag="cc_out", name=f"cco_m{l}{st}",
                                  addr_space="Shared")
                prb = evp.tile([128, KT * 512], BF16, tag="pr", bufs=1, name=f"prb{l}{st}")
            for dp in range(4):
                wdt = wdp.tile([128, 8 * 512], BF16, tag="wdt")
                nc.sync.dma_start(
                    wdt[:], _ap3(wd_in, l * FFS * DM + dp * 512,
                                 [[DM, 128], [128 * DM, 8], [1, 512]]))
                dps = [ps_mlp.tile([128, 512], F32, tag=f"mps{dmm}", name=f"mpsd{dmm}")
                       for dmm in range(4)]
                for fk in range(8):
                    for dmm in range(4):
                        nc.tensor.matmul(
                            dps[dmm][:],
                            wdt[:, fk * 512 + dmm * 128:fk * 512 + (dmm + 1) * 128],
                            gt[fk][:, sl], start=(fk == 0), stop=(fk == 7))
                if last:
                    prd = evp.tile([128, 4 * 512], F32, tag="prd", bufs=1,
                                   name=f"prd{dp}")
                for dmm in range(4):
                    kk = dp * 4 + dmm
                    if last:
                        nc.vector.scalar_tensor_tensor(
                            prd[:, dmm * 512:(dmm + 1) * 512], xt[kk][:, sl],
                            1.0 / NCORES, dps[dmm][:],
                            AluOpType.mult, AluOpType.add)
                    else:
                        nc.vector.tensor_copy(prb[:, kk * 512:(kk + 1) * 512],
                                              dps[dmm][:])
                if last:
                    nc.sync.dma_start(
                        _ap3(y_out, dp * 512 * S + st * 512,
                             [[S, 128], [128 * S, 4], [1, 512]]),
                        prd[:])
            ctx_mlp.close()
            if last:
                return None
            tgt = cc_out if no_ar else cc_in
            nc.sync.dma_start(
                _ap3(tgt[:], 0, [[512, 128], [128 * 512, KT], [1, 512]]),
                prb[:])
            if not no_ar:
                nc.gpsimd.collective_compute(
                    "AllReduce", AluOpType.add,
                    replica_groups=[list(range(NCORES))],
                    ins=[cc_in[:].opt()], outs=[cc_out[:].opt()])
            return cc_out

        def mlp_block(l, pending, last):
            gt = [actp.tile([128, S], BF16, tag=f"g{fi}", name=f"g{fi}")
                  for fi in range(8)]
            cc = []
            for st in range(2):
                mk(f"L{l}.mlp_st{st}")
                resid_add(pending[st], st)
                rb = rms_st(st)
                cc.append(mlp_st(l, st, last, gt, rb))
            return None if last else cc

        def dump_x():
            for k in range(KT):
                pr = evp.tile([128, S], F32, tag="dbg")
                nc.vector.tensor_copy(pr[:], xt[k][:])
                nc.sync.dma_start(y_out[k * 128:(k + 1) * 128, :], pr[:])

        for _ in range(reps):
            load_x()
            pending = None
            done = False
            for l in range(L):
                mk(f"attn{l}")
                pending = attn_block(l, pending)
                if debug_stage == f"attn{l}":
                    for st in range(2):
                        resid_add(pending[st], st)
                    dump_x()
                    done = True
                    break
                last = (l == L - 1 and debug_stage is None)
                mk(f"mlp{l}")
                pending = mlp_block(l, pending, last)
                if debug_stage == f"mlp{l}":
                    for st in range(2):
                        resid_add(pending[st], st)
                    dump_x()
                    done = True
                    break
            if done:
                break

    nc.compile()
    return nc


def make_inputs(input_ids, embed, wq, wk, wv, wo, wgate, wup, wdown, ln1, ln2):
    """host-side prep: embedding gather, shard + fold norm gains/scale into weights."""
    f32 = np.float32
    bf = ml_dtypes.bfloat16
    x = np.asarray(embed, f32)[np.asarray(input_ids)[0]]      # (S, DM)
    xt = np.ascontiguousarray(x.T).astype(bf)                 # (DM, S) bf16

    inv_freq = 1.0 / (THETA ** (np.arange(0, D, 2, dtype=f32) / D))
    freqs = np.arange(S, dtype=f32)[:, None] * inv_freq[None, :]    # (S, 32)
    emb = np.concatenate([freqs, freqs], axis=1)                    # (S, D)
    cosT = np.cos(emb).T.astype(f32)                                # (D, S)
    sinT = np.sin(emb).T.astype(f32)
    sinT_signed = sinT.copy()
    sinT_signed[: D // 2] *= -1.0
    cos_rep = np.concatenate([cosT, cosT], axis=0).astype(bf)       # (128, S) bf16
    sin_rep = np.concatenate([sinT_signed, sinT_signed], axis=0).astype(bf)

    mask = np.zeros((2, 128, 256), f32)
    jj = np.arange(128)[:, None]
    ii = np.arange(256)[None, :]
    for o in range(2):
        mask[o] = np.where(128 * o + jj <= ii, 0.0, NEG)

    scale_q = 1.0 / math.sqrt(D)
    in_maps = []
    for c in range(NCORES):
        wqkv = np.empty((L, DM, 384), f32)
        wo_c = np.empty((L, QC, DM), f32)
        wg_c = np.empty((L, DM, FFS), f32)
        wu_c = np.empty((L, DM, FFS), f32)
        wd_c = np.empty((L, FFS, DM), f32)
        for l in range(L):
            g1 = np.asarray(ln1[l], f32)[:, None]
            g2 = np.asarray(ln2[l], f32)[:, None]
            wqkv[l, :, :QC] = np.asarray(wq[l], f32)[:, c * QC:(c + 1) * QC] * g1 * scale_q
            wqkv[l, :, QC:QC + D] = np.asarray(wk[l], f32)[:, c * D:(c + 1) * D] * g1
            wqkv[l, :, QC + D:] = np.asarray(wv[l], f32)[:, c * D:(c + 1) * D] * g1
            wo_c[l] = np.asarray(wo[l], f32)[c * QC:(c + 1) * QC, :]
            wg_c[l] = np.asarray(wgate[l], f32)[:, c * FFS:(c + 1) * FFS] * g2
            wu_c[l] = np.asarray(wup[l], f32)[:, c * FFS:(c + 1) * FFS] * g2
            wd_c[l] = np.asarray(wdown[l], f32)[c * FFS:(c + 1) * FFS, :]
        in_maps.append({
            "xt": xt, "wqkv": wqkv.astype(bf),
            "wo": wo_c.astype(bf), "wg": wg_c.astype(bf), "wu": wu_c.astype(bf),
            "wd": wd_c.astype(bf), "cosr": cos_rep, "sinr": sin_rep, "mask": mask,
        })
    return in_maps


_NC_CACHE = {}


def kernel(**inputs) -> np.ndarray:
    if 1 not in _NC_CACHE:
        _NC_CACHE[1] = build(reps=1)
    nc = _NC_CACHE[1]
    in_maps = make_inputs(**inputs)
    res = run_bass_kernel_spmd(nc, in_maps, list(range(NCORES)))
    y = np.zeros((DM, S), np.float64)
    for c in range(NCORES):
        y += res.results[c]["y"].astype(np.float64)
    return np.ascontiguousarray(y.T.astype(np.float32)).reshape(B, S, DM)
